# revision 20
# baseline (speedup 1.0000x reference)
"""GCNConv on 8 Trainium2 NeuronCores.

out = in_norm * (A @ (out_norm * (x @ W))) + bias, A = unweighted CSR adjacency.

Design (v2, superstep-batched fp16):
- Each core owns 1/8 of the destination rows and holds the FULL x in its own
  HBM as an fp16 table packed 4 nodes per row ([25000, 512]) so int16
  dma_gather indices reach all 100k nodes via 4 residue-bucketed calls.
- Gathers are batched per SUPERSTEP of SS dest-windows (SS*128 dests,
  SS*2048 edges): one dma_gather per (superstep, residue) instead of per
  (window, residue). The SWDGE Q7 fixed cost (~1us/call) drops ~10x and
  per-window 128-padding of residue buckets disappears (only %16 pad).
- Buckets keep CSR (dest-sorted) order, so each window's slots form a
  contiguous range; groups of 128 slots at bucket-global alignment feed the
  PE selection matmuls. Boundary groups shared by two windows are matmul'd
  twice with S masked per window: S = is_equal(did, iota + 128*w_local)
  where did holds superstep-local dest ids (sentinel 2047 for pad slots).
- All matmuls fp16 (1 cyc/row on PE vs 4 for fp32): aggregation
  psum += g_group^T-free @ S, then bias via ones x bias matmul accumulated
  under the weight matmul, PSUM->SBUF copies on the Activation engine.
- Degree norms fold into W when degrees are uniform (they are here: deg=16).
- Bucket sizes differ per core, so each core gets its own NEFF; the 8
  single-device executables run concurrently via PJRT.
"""
import math
import os as _os
import numpy as np

import jax

import concourse.bass as bass
import concourse.bacc as bacc
import concourse.mybir as mybir
from concourse.tile import TileContext
from concourse.bass2jax import (
    _bass_exec_p, install_neuronx_cc_hook, partition_id_tensor,
)

N_CORES = 8
C = 128
P = 128
f32 = mybir.dt.float32
f16 = mybir.dt.float16
i16 = mybir.dt.int16

SENT = 2047.0               # superstep-local dest sentinel (fp16-exact, >= SS*128)
SS = int(_os.environ.get("GCN_SS", "12"))        # windows per superstep
GP_BUFS = int(_os.environ.get("GCN_GP_BUFS", "2"))
SP_BUFS = int(_os.environ.get("GCN_SP_BUFS", "3"))
OUTB = _os.environ.get("GCN_OUTB", "1") == "1"   # batched per-superstep out DMA
TSB = _os.environ.get("GCN_TSB", "1") == "1"     # tensor_scalar S-build
CH = int(_os.environ.get("GCN_CH", "8"))         # groups per gather call
# (8 groups = 1024 descriptors = the SWDGE ring capacity; >1024 descs per
# call crashes the device - HW-bisected)

_CACHE = {}


def _wrap_idx(idx):
    """[n] int (n%16==0) -> [128, n/16] int16 wrapped + replicated."""
    w = np.asarray(idx, np.int16).reshape(-1, 16).T
    return np.ascontiguousarray(np.tile(w, (8, 1)))


def _prep_core(c, n_dest, rowptr, colind, v_edge):
    """Host-side metadata for core c.

    Returns list of supersteps; each is (buckets, w0, nwin_ss) with
    buckets[r] = (qp, dp, vp, b, G, spans):
      qp [n16] int idx (table row, -1 pad), dp [G*128] superstep-local dest
      (SENT pad), vp [G*128] edge weight or None, b real edges, G groups,
      spans[wl] = (slot_a, slot_b) of window wl's edges in this bucket.
    """
    d0 = c * n_dest
    nwin = math.ceil(n_dest / P)
    out = []
    for s0 in range(0, nwin, SS):
        nwin_ss = min(SS, nwin - s0)
        wd0 = d0 + s0 * P
        wd1 = min(wd0 + nwin_ss * P, d0 + n_dest)
        e0, e1 = int(rowptr[wd0]), int(rowptr[wd1])
        srcs = colind[e0:e1].astype(np.int64)
        dloc = np.searchsorted(rowptr[wd0:wd1 + 1] - rowptr[wd0],
                               np.arange(e1 - e0), side="right") - 1
        vv = v_edge[e0:e1] if v_edge is not None else None
        res = srcs & 3
        q = srcs >> 2
        buckets = []
        for r in range(4):
            m = res == r
            dr, qr = dloc[m], q[m]           # CSR order: dr non-decreasing
            vr = vv[m] if vv is not None else None
            b = len(qr)
            G = max(1, (b + P - 1) // P)
            # pad to full groups with idx 0 (row 0 gathered, did=sentinel):
            # every slot is written each superstep -> no stale/NaN data
            qp = np.zeros(G * P, np.int64)
            qp[:b] = qr
            dp = np.full(G * P, SENT, np.float32)
            dp[:b] = dr
            vp = None
            if vr is not None:
                vp = np.zeros(G * P, np.float32)
                vp[:b] = vr
            spans = []
            for wl in range(nwin_ss):
                a = int(np.searchsorted(dr, wl * P, side="left"))
                bb = int(np.searchsorted(dr, (wl + 1) * P, side="left"))
                spans.append((a, bb))
            buckets.append((qp, dp, vp, b, G, spans))
        out.append((buckets, s0, nwin_ss))
    return out


def _build_core(n_dest, n_table_rows, supersteps, uniform, nq=4, repeat=1):
    """Build one core's Bacc kernel."""
    idx_parts, did_parts, v_parts = [], [], []
    cum16 = 0      # idx slots (16-mult)
    cumd = 0       # did slots (128-mult)
    meta = []      # per ss: (col0s[r], n16s[r], bs[r], dslot0s[r], Gs[r])
    Gtot_max = 0
    span_max = 1
    for (buckets, s0, nwin_ss) in supersteps:
        cols, n16s, bs, dcol0s, Gs = [], [], [], [], []
        Gtot = 0
        for (qp, dp, vp, b, G, spans) in buckets:
            cols.append(cum16 // 16)
            n16s.append(len(qp))
            bs.append(b)
            dcol0s.append(cumd // P)
            Gs.append(G)
            cum16 += len(qp)
            cumd += G * P
            Gtot += G
            idx_parts.append(qp)
            did_parts.append(dp)
            if vp is not None:
                v_parts.append(vp)
            for (a, bb) in spans:
                ga, gb = a // P, (bb + P - 1) // P
                span_max = max(span_max, max(1, gb - ga))
        meta.append((cols, n16s, bs, dcol0s, Gs))
        Gtot_max = max(Gtot_max, Gtot)
    idx_all = np.concatenate(idx_parts)
    did_all = np.concatenate(did_parts)
    tot_cols = len(idx_all) // 16
    dcols = len(did_all) // P
    ssw = max(nw for (_, _, nw) in supersteps)

    nc = bacc.Bacc("TRN2", target_bir_lowering=False, num_devices=1,
                   num_swdge_queues=nq)
    x = nc.dram_tensor("x", [n_table_rows, 4 * C], f16, kind="ExternalInput")
    idxd = nc.dram_tensor("idx", [128, tot_cols], i16, kind="ExternalInput")
    didd = nc.dram_tensor("did", [128, dcols], f32, kind="ExternalInput")
    wtd = nc.dram_tensor("wt", [C, C], f32, kind="ExternalInput")
    biasd = nc.dram_tensor("biasb", [128, C], f32, kind="ExternalInput")
    vd = None
    if not uniform:
        vd = nc.dram_tensor("v", [128, dcols], f32, kind="ExternalInput")
    outd = nc.dram_tensor("out", [n_dest, C], f32, kind="ExternalOutput")

    with TileContext(nc) as tc:
        with tc.tile_pool(name="const", bufs=1) as cp, \
             tc.tile_pool(name="gp", bufs=GP_BUFS) as gp, \
             tc.tile_pool(name="sp", bufs=SP_BUFS) as spool, \
             tc.tile_pool(name="op", bufs=2) as op, \
             tc.tile_pool(name="ps", bufs=6, space="PSUM") as ps, \
             tc.tile_pool(name="ps2", bufs=2, space="PSUM") as ps2:
            idx_t = cp.tile([128, tot_cols], i16, name="idxt")
            nc.sync.dma_start(idx_t[:], idxd[:])
            did_t = cp.tile([128, dcols], f32, name="didt")
            nc.sync.dma_start(did_t[:], didd[:])
            wt_t = cp.tile([C, C], f32, name="wtt")
            nc.sync.dma_start(wt_t[:], wtd[:])
            wt16_t = cp.tile([C, C], f16, name="wt16t")
            nc.scalar.copy(out=wt16_t[:], in_=wt_t[:])
            bias_t = cp.tile([128, C], f32, name="biast")
            nc.sync.dma_start(bias_t[:], biasd[:])
            bias16_t = cp.tile([128, C], f16, name="bias16t")
            nc.scalar.copy(out=bias16_t[:], in_=bias_t[:])
            ones_t = cp.tile([128, C], f16, name="onest")
            nc.vector.memset(ones_t[:], 1.0 / 128.0)
            # iota_kw[p, w*128 + j] = w*128 + j  (fp16-exact: < 2048)
            iota_t = cp.tile([128, ssw * 128], f16, name="iotat")
            nc.gpsimd.iota(iota_t[:], pattern=[[1, ssw * 128]], base=0,
                           channel_multiplier=0,
                           allow_small_or_imprecise_dtypes=True)
            v_t = None
            if vd is not None:
                v_t = cp.tile([128, dcols], f32, name="vt")
                nc.sync.dma_start(v_t[:], vd[:])

            # global SWDGE call counter: Tile assigns DMASW sem lane k%8 to
            # the k-th SWDGE DMA in program order, and a lane must stay on
            # one queue -> queue k%nq (nq divides 8) keeps lane/queue pairs
            # consistent
            qn = 0
            for rep in range(repeat):
              for ssi, (buckets, s0, nwin_ss) in enumerate(supersteps):
                cols, n16s, bs, dcol0s, Gs = meta[ssi]
                g = gp.tile([128, Gtot_max, C], f16,
                            name=f"g{rep}_{ssi}", tag="g")
                gofs = [0, 0, 0, 0]
                acc = 0
                for r in range(4):
                    gofs[r] = acc
                    G = Gs[r]
                    for g0 in range(0, G, CH):
                        gn = min(CH, G - g0)
                        nc.gpsimd.dma_gather(
                            g[:, acc + g0:acc + g0 + gn, :],
                            x[:, r * C:(r + 1) * C],
                            idx_t[:, cols[r] + g0 * 8:
                                  cols[r] + (g0 + gn) * 8],
                            gn * P, gn * P, C, elem_step=4 * C,
                            queue_num=qn % nq,
                        )
                        qn += 1
                    acc += G
                osbB = op.tile([128, nwin_ss, 128], f32,
                               name=f"osbB{rep}_{ssi}", tag="osb")
                for wl in range(nwin_ss):
                    w = s0 + wl
                    psum = ps.tile([128, 128], f32, name=f"ps{rep}_{w}",
                                   tag="psw", space="PSUM")
                    mms = []
                    for r in range(4):
                        (a, bb) = supersteps[ssi][0][r][5][wl]
                        if bb <= a:
                            continue
                        ga, gb = a // P, (bb + P - 1) // P
                        span = gb - ga
                        s_t = spool.tile([128, span_max, 128], f16,
                                         name=f"s{rep}_{w}_{r}", tag=f"s{r}")
                        isl = iota_t[:, wl * 128:(wl + 1) * 128]
                        if TSB:
                            for gi in range(ga, gb):
                                # S[slot, j] = (did[slot] == wl*128 + j); all
                                # operands packed-last -> DVE fast mode
                                nc.vector.tensor_scalar(
                                    out=s_t[:, gi - ga, :], in0=isl,
                                    scalar1=did_t[:, dcol0s[r] + gi:
                                                  dcol0s[r] + gi + 1],
                                    scalar2=None,
                                    op0=mybir.AluOpType.is_equal)
                                if v_t is not None:
                                    nc.vector.tensor_scalar(
                                        out=s_t[:, gi - ga, :],
                                        in0=s_t[:, gi - ga, :],
                                        scalar1=v_t[:, dcol0s[r] + gi:
                                                    dcol0s[r] + gi + 1],
                                        scalar2=None,
                                        op0=mybir.AluOpType.mult)
                        else:
                            dslice = did_t[:, dcol0s[r] + ga:dcol0s[r] + gb]
                            din = bass.AP(dslice.tensor, dslice.offset,
                                          dslice.ap + [(0, 128)])
                            iin = bass.AP(isl.tensor, isl.offset,
                                          [isl.ap[0], (0, span), isl.ap[1]])
                            nc.vector.tensor_tensor(
                                out=s_t[:, :span, :], in0=din, in1=iin,
                                op=mybir.AluOpType.is_equal)
                            if v_t is not None:
                                vslice = v_t[:, dcol0s[r] + ga:dcol0s[r] + gb]
                                vin = bass.AP(vslice.tensor, vslice.offset,
                                              vslice.ap + [(0, 128)])
                                nc.vector.tensor_tensor(
                                    out=s_t[:, :span, :],
                                    in0=s_t[:, :span, :],
                                    in1=vin, op=mybir.AluOpType.mult)
                        for gi in range(ga, gb):
                            mms.append((gofs[r] + gi, s_t, gi - ga))
                    for k, (gcol, s_t, scol) in enumerate(mms):
                        nc.tensor.matmul(
                            out=psum[:],
                            lhsT=g[:, gcol, :],
                            rhs=s_t[:, scol, :],
                            start=(k == 0), stop=(k == len(mms) - 1))
                    aggrT = op.tile([128, 128], f16, name=f"aggrT{rep}_{w}",
                                    tag="aggrT")
                    nc.scalar.copy(out=aggrT[:], in_=psum[:])
                    psum2 = ps2.tile([128, 128], f32, name=f"q{rep}_{w}",
                                     tag="psq", space="PSUM")
                    nc.tensor.matmul(out=psum2[:], lhsT=ones_t[:],
                                     rhs=bias16_t[:], start=True, stop=False)
                    nc.tensor.matmul(out=psum2[:], lhsT=aggrT[:],
                                     rhs=wt16_t[:], start=False, stop=True)
                    nc.scalar.copy(out=osbB[:, wl, :], in_=psum2[:])
                    if not OUTB:
                        d0, d1 = w * P, min((w + 1) * P, n_dest)
                        nc.sync.dma_start(outd[d0:d1, :],
                                          osbB[:d1 - d0, wl, :])
                if OUTB:
                    # one batched output DMA per superstep: full windows in
                    # one transposed-AP write, trailing partial separately
                    nfull = nwin_ss
                    d_end = min((s0 + nwin_ss) * P, n_dest)
                    partial = (s0 + nwin_ss) * P - d_end
                    if partial:
                        nfull -= 1
                    if nfull:
                        base = outd[s0 * P:s0 * P + nfull * P, :]
                        dview = bass.AP(base.tensor, base.offset,
                                        [(C, P), (P * C, nfull), (1, C)])
                        nc.sync.dma_start(dview, osbB[:, :nfull, :])
                    if partial:
                        d0 = (s0 + nfull) * P
                        nc.sync.dma_start(outd[d0:d_end, :],
                                          osbB[:d_end - d0, nfull, :])
    nc.compile()
    v_all = np.concatenate(v_parts) if v_parts else None
    return nc, idx_all, did_all, v_all


def _make_single_runner(nc):
    install_neuronx_cc_hook()
    pname = nc.partition_id_tensor.name if nc.partition_id_tensor else None
    in_names, out_names, out_avals, zero_outs = [], [], [], []
    for alloc in nc.m.functions[0].allocations:
        if not isinstance(alloc, mybir.MemoryLocationSet):
            continue
        name = alloc.memorylocations[0].name
        if alloc.kind == "ExternalInput":
            if name != pname:
                in_names.append(name)
        elif alloc.kind == "ExternalOutput":
            shape = tuple(alloc.tensor_shape)
            dtype = mybir.dt.np(alloc.dtype)
            out_avals.append(jax.core.ShapedArray(shape, dtype))
            zero_outs.append(np.zeros(shape, dtype))
            out_names.append(name)
    all_in = list(in_names) + list(out_names)
    if pname is not None:
        all_in.append(pname)

    def _body(*args):
        operands = list(args)
        if pname is not None:
            operands.append(partition_id_tensor())
        return tuple(_bass_exec_p.bind(
            *operands, out_avals=tuple(out_avals), in_names=tuple(all_in),
            out_names=tuple(out_names),
            lowering_input_output_aliases=(),
            sim_require_finite=True, sim_require_nnan=True, nc=nc))

    fn = jax.jit(_body, keep_unused=True)
    return fn, in_names, out_names, zero_outs


def _build_all(x, weight, bias, rowptr, colind, colptr):
    n_nodes = rowptr.shape[0] - 1
    n_dest = n_nodes // N_CORES

    deg_in = np.diff(rowptr).astype(np.float64)
    deg_out = np.diff(colptr).astype(np.float64)
    with np.errstate(divide="ignore"):
        in_norm = 1.0 / np.sqrt(deg_in)
        out_norm = 1.0 / np.sqrt(deg_out)
    n_used = min(colind.shape[0], int(rowptr[-1]))
    uniform = bool(np.all(deg_in == deg_in[0]) and np.all(deg_out == deg_out[0])
                   and np.isfinite(in_norm[0]) and np.isfinite(out_norm[0]))
    if uniform:
        v_edge = None
        w_eff = (weight * np.float32(in_norm[0] * out_norm[0])).astype(np.float32)
    else:
        row_of_edge = (np.searchsorted(rowptr, np.arange(n_used),
                                       side="right") - 1)
        v_edge = (out_norm[colind[:n_used]] * in_norm[row_of_edge]).astype(
            np.float32)
        w_eff = weight.astype(np.float32)

    n_pad_rows = ((n_nodes + 3) // 4) * 4
    n_table_rows = n_pad_rows // 4
    assert n_table_rows <= 32768, "int16 gather reach exceeded"

    bias_b = np.ascontiguousarray(
        np.tile(bias[None, :], (128, 1)).astype(np.float32))

    cores = []
    for c in range(N_CORES):
        supersteps = _prep_core(c, n_dest, rowptr, colind[:n_used], v_edge)
        nc, idx_all, did_all, v_all = _build_core(
            n_dest, n_table_rows, supersteps, uniform)
        fn, in_names, out_names, zero_outs = _make_single_runner(nc)
        in_map = {
            "idx": _wrap_idx(idx_all),
            "did": np.ascontiguousarray(
                did_all.reshape(-1, 128).T.astype(np.float32)),
            "wt": w_eff,
            "biasb": bias_b,
        }
        if v_all is not None:
            in_map["v"] = np.ascontiguousarray(
                v_all.reshape(-1, 128).T.astype(np.float32))
        cores.append((fn, in_names, out_names, zero_outs, in_map))
    return cores, n_pad_rows, deg_in


def get_runners(x, weight, bias, rowptr, colind, colptr):
    key = (x.shape, hash(rowptr.tobytes()), hash(colind.tobytes()),
           hash(colptr.tobytes()))
    if key not in _CACHE:
        _CACHE[key] = _build_all(x, weight, bias, rowptr, colind, colptr)
    return _CACHE[key]


def run_on_device(cores, x_view, bias, deg_in):
    futs = []
    for c, (fn, in_names, out_names, zero_outs, in_map) in enumerate(cores):
        dev = jax.devices()[c]
        full = dict(in_map, x=x_view)
        dev_in = [jax.device_put(np.asarray(full[n]), dev) for n in in_names]
        dev_zero = [jax.device_put(z, dev) for z in zero_outs]
        futs.append((fn(*dev_in, *dev_zero), out_names))
    results = []
    for (out_arrs, out_names) in futs:
        jax.block_until_ready(out_arrs)
        results.append(np.asarray(out_arrs[out_names.index("out")]))
    out = np.concatenate(results, axis=0)
    zero_deg = deg_in == 0
    if zero_deg.any():
        out[zero_deg] = (np.float32(0) * np.float32(np.inf)) + bias[None, :]
    return out


def kernel(x, weight, bias, rowptr, colind, colptr, rowind):
    x = np.ascontiguousarray(np.asarray(x, np.float32))
    weight = np.asarray(weight, np.float32)
    bias = np.asarray(bias, np.float32)
    rowptr = np.asarray(rowptr, np.int64)
    colind = np.asarray(colind, np.int64)
    colptr = np.asarray(colptr, np.int64)

    n_nodes = rowptr.shape[0] - 1
    cores, n_pad_rows, deg_in = get_runners(x, weight, bias, rowptr, colind,
                                            colptr)
    if n_pad_rows == n_nodes:
        x_view = np.ascontiguousarray(x.astype(np.float16)).reshape(
            n_nodes // 4, 4 * C)
    else:
        xp = np.zeros((n_pad_rows, C), np.float16)
        xp[:n_nodes] = x
        x_view = xp.reshape(n_pad_rows // 4, 4 * C)
    return run_on_device(cores, x_view, bias, deg_in)
